# revision 17
# baseline (speedup 1.0000x reference)
"""Trainium2 Bass kernel for NeuroISNet GNN message passing (v2).

Strategy (8 NeuronCores, one trn2 chip):
  - Batch b -> core pair (2b, 2b+1); each core owns 2048 of 4096 node rows.
  - Dominant bmm msg = x @ m runs in fp8e4 with perf_mode=DoubleRow:
    x^T resident in SBUF as 16 pair-chunks [128, 2, 2048] (8 MB), m in fp8
    pair tiles; 256-deep contraction per matmul, f32 PSUM accumulation.
  - LSTM gates also fp8 DoubleRow: one matmul per gate computes
    Wih@msg + Whh@h via the two planes of a shared [128, 2, 512] rhs tile
    (plane0 = 4*msg, plane1 = 2*h). Weights stored x16/x32 so fp8 keeps
    mantissa; activation `scale` undoes the folding.
  - All-tanh LSTM: sigmoid(x) = (tanh(x/2)+1)/2, cell state stored as
    C = 2c and hidden as Hs = 2h (LN is scale-invariant; Whh absorbs the
    0.5) -> the scalar engine runs ONLY Tanh, zero act-table reloads.
  - LayerNorm per 128-row tile via PE transpose + bn_stats; 1/sigma via
    quake-rsqrt (bitcast + one Newton step) on the vector engine.
  - msg-MLP runs on LOCAL 2048 nodes only; the m output (fp8) is
    AllGathered between the core pair as two chunked collectives per
    iteration, issued mid-iteration so the bmm of the next iteration
    never waits. m-chunk availability is rank-symmetric (both halves of
    each gather land together), so the SPMD program has no rank branches.
  - m3's bias enters the bmm as a rank-1 matmul with x row-sums.
  - Iteration 1 exploits identical initial rows: msg1 = m0 (x) rowsums.
  - Host does only O(B*H^2) prep: folding weights, init MLP, transposes.
"""

import numpy as np
import ml_dtypes

import concourse.bass as bass
import concourse.mybir as mybir
import concourse.tile as tile
from concourse import bacc
from concourse.bass_utils import run_bass_kernel_spmd

BF = ml_dtypes.bfloat16
F8 = ml_dtypes.float8_e4m3
bf16 = mybir.dt.bfloat16
f32 = mybir.dt.float32
fp8 = mybir.dt.float8e4
u32 = mybir.dt.uint32

B, N, H, ITERS = 4, 4096, 128, 8
EPS = 1e-5
NCORES = 8
R = N // 2               # local rows per core
NRB = 4                  # row blocks
RB = R // NRB            # 512
NPAIR = 16               # global DoubleRow pairs (4096 / 256)
NLCH = 16                # local 128-chunks (2048 / 128)
GROUPS = [[0, 1], [2, 3], [4, 5], [6, 7]]

AF = mybir.ActivationFunctionType
ALU = mybir.AluOpType
PM = mybir.MatmulPerfMode

WIH_S = 16.0             # lstm_wih stored x16 (fp8 range)
WHH_S = 32.0             # lstm_whh stored x32 (fp8 range + absorbs h = Hs/2)
M_S = 4.0                # m stored x4 (fp8 subnormal floor)
GS = WIH_S * M_S         # psum gate scale (= WHH_S * 2)
QUAKE_K = 0x5F3759DF


def build_module(iters=ITERS):
    nc = bacc.Bacc("TRN2", target_bir_lowering=False, debug=False,
                   num_devices=NCORES)

    din = lambda name, shape, dt: nc.dram_tensor(name, shape, dt,
                                                 kind="ExternalInput")
    xtdr_in = din("xtdr", [NPAIR * 128, 2, R], fp8)
    h0_in = din("h0", [H, R], fp8)              # Hs = 2*h0
    rs_in = din("rs", [1, R], bf16)
    m0_in = din("m0", [1, H], bf16)             # 4*m0eff
    b3r_in = din("b3r", [1, H], bf16)           # 4*msg_b3
    w1gt_in = din("w1gt", [H, H], bf16)
    w2t_in = din("w2t", [H, H], bf16)
    w3t4_in = din("w3t4", [H, H], bf16)         # 4*msg_w3.T
    vw1gt_in = din("vw1gt", [H, H], bf16)
    vw2t_in = din("vw2t", [H, H], bf16)
    vw3t_in = din("vw3t", [H, 1], bf16)
    wpair_in = din("wpair", [H, 2, 4 * H], fp8)  # [16*wih.T | 32*whh.T]
    bg2_in = din("bg2", [H, 4], f32)            # pre-scaled gate biases
    b1c_in = din("b1c", [H, 1], f32)
    b2c_in = din("b2c", [H, 1], f32)
    vb1c_in = din("vb1c", [H, 1], f32)
    vb2c_in = din("vb2c", [H, 1], f32)
    id8_in = din("id8", [H, H], fp8)
    id16_in = din("id16", [H, H], bf16)

    votes_out = nc.dram_tensor("votes", [1, R], f32, kind="ExternalOutput")

    rbsl = lambda rb: slice(rb * RB, (rb + 1) * RB)

    with tile.TileContext(nc) as tc:
        with tc.tile_pool(name="const", bufs=1) as cp, \
             tc.tile_pool(name="state", bufs=1) as st, \
             tc.tile_pool(name="work", bufs=1) as wk, \
             tc.tile_pool(name="ps", bufs=1, space="PSUM") as ps, \
             tc.tile_pool(name="dram", bufs=1, space="DRAM") as dr:

            # ---- constants ----
            def cload(inp, shape, dt, tag):
                t = cp.tile(shape, dt, tag=tag, name=tag)
                nc.sync.dma_start(t[:], inp[:])
                return t

            w1gt = cload(w1gt_in, [H, H], bf16, "w1gt")
            w2t = cload(w2t_in, [H, H], bf16, "w2t")
            w3t4 = cload(w3t4_in, [H, H], bf16, "w3t4")
            vw1gt = cload(vw1gt_in, [H, H], bf16, "vw1gt")
            vw2t = cload(vw2t_in, [H, H], bf16, "vw2t")
            vw3t = cload(vw3t_in, [H, 1], bf16, "vw3t")
            wpair = cload(wpair_in, [H, 2, 4 * H], fp8, "wpair")
            bg2 = cload(bg2_in, [H, 4], f32, "bg2")
            b1c = cload(b1c_in, [H, 1], f32, "b1c")
            b2c = cload(b2c_in, [H, 1], f32, "b2c")
            vb1c = cload(vb1c_in, [H, 1], f32, "vb1c")
            vb2c = cload(vb2c_in, [H, 1], f32, "vb2c")
            id8 = cload(id8_in, [H, H], fp8, "id8")
            id16 = cload(id16_in, [H, H], bf16, "id16")
            rs_sb = cload(rs_in, [1, R], bf16, "rs")
            m0_sb = cload(m0_in, [1, H], bf16, "m0")
            b3r = cload(b3r_in, [1, H], bf16, "b3r")
            qk = cp.tile([H, 4], u32, tag="qk", name="qk")
            nc.vector.memset(qk[:], QUAKE_K)

            # ---- state ----
            grhs = []                 # gates rhs: plane0 = 4*msg, plane1 = Hs
            for rb in range(NRB):
                t = st.tile([H, 2, RB], fp8, tag=f"grhs{rb}", name=f"grhs{rb}")
                nc.sync.dma_start(t[:, 1, :], h0_in[:, rbsl(rb)])
                grhs.append(t)
            cst = []                  # C = 2c, bf16
            for rb in range(NRB):
                t = st.tile([H, RB], bf16, tag=f"c{rb}", name=f"c{rb}")
                nc.vector.memset(t[:], 0.0)
                cst.append(t)
            hnLoc = st.tile([H, R], bf16, tag="hnLoc", name="hnLoc")

            # ---- resident x^T pair-chunks, in bmm consumption order ----
            ORDER_A = [0, 1, 2, 3, 8, 9, 10, 11]
            ORDER_B = [4, 5, 6, 7, 12, 13, 14, 15]
            xt = [None] * NPAIR
            for c in ORDER_A + ORDER_B:
                t = st.tile([128, 2, R], fp8, tag=f"xt{c}", name=f"xt{c}")
                nc.sync.dma_start(t[:], xtdr_in[c * 128:(c + 1) * 128, :, :])
                xt[c] = t

            def pair_lhsT(c, mfA, mfB):
                """lhsT [128, 2, H] for global pair c from gathered m tiles."""
                if c < 8:
                    t, s = (mfA, 2 * c) if c < 4 else (mfB, 2 * (c - 4))
                else:
                    t, s = (mfA, 8 + 2 * (c - 8)) if c < 12 else \
                           (mfB, 8 + 2 * (c - 12))
                return t[:, s:s + 2, :]

            mf_cur = (None, None)

            # warm up the collective path while input DMAs stream
            cinw = dr.tile([1, 16], fp8, tag="cinw", name="cinw")
            coutw = dr.tile([2, 16], fp8, tag="coutw", name="coutw")
            nc.sync.dma_start(cinw[:], id8[0:1, 0:16])
            nc.gpsimd.collective_compute(
                "AllGather", ALU.bypass, replica_groups=GROUPS,
                ins=[cinw[:].opt()], outs=[coutw[:].opt()])

            # ================= main loop =================
            # rb-major software pipeline: each rb's bmm block (A_rb) is
            # emitted between the previous rb's dependent tail work, so the
            # PE always has ready matmuls while scalar/vector chains run.
            mf_cur = (None, None)

            def tail_chain(it, rb, gps):
                """LSTM tail: scalar tanhs + gpsimd/vector state update."""
                gi, gf, gg, go = gps
                ti = wk.tile([H, RB], bf16, tag="ti", bufs=2,
                             name=f"ti_{it}_{rb}")
                nc.scalar.activation(ti[:], gi[:], AF.Tanh,
                                     bias=bg2[:, 0:1], scale=0.5 / GS)
                tf = wk.tile([H, RB], bf16, tag="tf", bufs=2,
                             name=f"tf_{it}_{rb}")
                nc.scalar.activation(tf[:], gf[:], AF.Tanh,
                                     bias=bg2[:, 1:2], scale=0.5 / GS)
                tg = wk.tile([H, RB], bf16, tag="tg", bufs=2,
                             name=f"tg_{it}_{rb}")
                nc.scalar.activation(tg[:], gg[:], AF.Tanh,
                                     bias=bg2[:, 2:3], scale=1.0 / GS)
                to = wk.tile([H, RB], bf16, tag="to", bufs=2,
                             name=f"to_{it}_{rb}")
                nc.scalar.activation(to[:], go[:], AF.Tanh,
                                     bias=bg2[:, 3:4], scale=0.5 / GS)
                # C' = 0.5*(tf+1)*C + (ti+1)*tg   (C = 2c)
                uu = wk.tile([H, RB], bf16, tag="uu", bufs=2,
                             name=f"uu_{it}_{rb}")
                nc.vector.scalar_tensor_tensor(
                    uu[:], tf[:], 1.0, cst[rb][:], ALU.add, ALU.mult)
                vv = wk.tile([H, RB], bf16, tag="vv", bufs=2,
                             name=f"vv_{it}_{rb}")
                nc.vector.scalar_tensor_tensor(
                    vv[:], ti[:], 1.0, tg[:], ALU.add, ALU.mult)
                nc.vector.scalar_tensor_tensor(
                    cst[rb][:], uu[:], 0.5, vv[:], ALU.mult, ALU.add)
                tnc = wk.tile([H, RB], bf16, tag="tnc", bufs=2,
                              name=f"tnc_{it}_{rb}")
                nc.scalar.activation(tnc[:], cst[rb][:], AF.Tanh, scale=0.5)
                # Hs = (to+1)*tanh(c)   (= 2h)
                nc.vector.scalar_tensor_tensor(
                    grhs[rb][:, 1, :], to[:], 1.0, tnc[:], ALU.add, ALU.mult)

            def tail_pe(it, rb, mloc, mfA_n, mfB_n):
                """LN + local msg-MLP (or vote) for one row block."""
                # fp8 transpose must write with element step 2
                trp = ps.tile([128, 2 * RB], fp8, tag="ptrp", bufs=1,
                              name=f"trp_{it}_{rb}")
                tsl8 = lambda t: slice(2 * t * 128, 2 * (t + 1) * 128, 2)
                mv = wk.tile([128, 8], f32, tag="mv", bufs=2,
                             name=f"mv_{it}_{rb}")
                for t in range(4):
                    nc.tensor.transpose(trp[:, tsl8(t)],
                                        grhs[rb][:, 1, t * 128:(t + 1) * 128],
                                        id8[:])
                    stt = wk.tile([128, 6], f32, tag="st6", bufs=4,
                                  name=f"st_{it}_{rb}_{t}")
                    nc.vector.bn_stats(stt[:], trp[:, tsl8(t)])
                    nc.vector.bn_aggr(mv[:, 2 * t:2 * t + 2], stt[:])
                # quake rsqrt(var + eps), one Newton step
                ve = wk.tile([128, 4], f32, tag="ve", bufs=2,
                             name=f"ve_{it}_{rb}")
                nc.vector.tensor_scalar(ve[:], mv[:, 1:8:2], EPS, None, ALU.add)
                sh = wk.tile([128, 4], u32, tag="sh", bufs=2,
                             name=f"sh_{it}_{rb}")
                nc.vector.tensor_scalar(sh[:], ve[:].bitcast(u32), 1, None,
                                        ALU.logical_shift_right)
                y0 = wk.tile([128, 4], u32, tag="y0", bufs=2,
                             name=f"y0_{it}_{rb}")
                nc.vector.tensor_tensor(y0[:], qk[:], sh[:], ALU.subtract)
                y0f = y0[:].bitcast(f32)
                q2 = wk.tile([128, 4], f32, tag="q2", bufs=2,
                             name=f"q2_{it}_{rb}")
                nc.vector.tensor_tensor(q2[:], y0f, y0f, ALU.mult)
                nc.vector.tensor_tensor(q2[:], q2[:], ve[:], ALU.mult)
                nc.vector.tensor_scalar(q2[:], q2[:], -0.5, 1.5,
                                        ALU.mult, ALU.add)
                sc4 = wk.tile([128, 4], f32, tag="sc4", bufs=2,
                              name=f"sc4_{it}_{rb}")
                nc.vector.tensor_tensor(sc4[:], q2[:], y0f, ALU.mult)
                # apply + transpose back
                hnr = wk.tile([128, RB], bf16, tag="hnr", bufs=2,
                              name=f"hnr_{it}_{rb}")
                for t in range(4):
                    nc.vector.tensor_scalar(
                        hnr[:, t * 128:(t + 1) * 128], trp[:, tsl8(t)],
                        mv[:, 2 * t:2 * t + 1], sc4[:, t:t + 1],
                        ALU.subtract, ALU.mult)
                cbk = ps.tile([128, RB], bf16, tag="pbk", bufs=1,
                              name=f"cbk_{it}_{rb}")
                for t in range(4):
                    nc.tensor.transpose(cbk[:, t * 128:(t + 1) * 128],
                                        hnr[:, t * 128:(t + 1) * 128],
                                        id16[:])
                nc.vector.tensor_copy(hnLoc[:, rbsl(rb)], cbk[:])

                if it < iters:
                    m1p = ps.tile([H, RB], f32, tag="pmlp", bufs=1,
                                  name=f"m1p_{it}_{rb}")
                    nc.tensor.matmul(m1p[:], w1gt[:], hnLoc[:, rbsl(rb)],
                                     start=True, stop=True)
                    m1s = wk.tile([H, RB], bf16, tag="m1s", bufs=2,
                                  name=f"m1s_{it}_{rb}")
                    nc.vector.tensor_scalar(m1s[:], m1p[:], b1c[:], 0.0,
                                            ALU.add, ALU.max)
                    m2p = ps.tile([H, RB], f32, tag="pmlp", bufs=1,
                                  name=f"m2p_{it}_{rb}")
                    nc.tensor.matmul(m2p[:], w2t[:], m1s[:],
                                     start=True, stop=True)
                    m2s = wk.tile([H, RB], bf16, tag="m2s", bufs=2,
                                  name=f"m2s_{it}_{rb}")
                    nc.vector.tensor_scalar(m2s[:], m2p[:], b2c[:], 0.0,
                                            ALU.add, ALU.max)
                    m3p = ps.tile([H, RB], f32, tag="pmlp", bufs=1,
                                  name=f"m3p_{it}_{rb}")
                    for t in range(4):
                        nc.tensor.matmul(m3p[:, t * 128:(t + 1) * 128],
                                         m2s[:, t * 128:(t + 1) * 128],
                                         w3t4[:], start=True, stop=True)
                    nc.vector.tensor_copy(mloc[:, rb * 4:(rb + 1) * 4, :],
                                          m3p[:])
                    if rb == 1:
                        cin0 = dr.tile([128, 8, H], fp8, tag="cin0",
                                       bufs=2, name=f"cin0_{it}")
                        cout0 = dr.tile([256, 8, H], fp8, tag="cout0",
                                        bufs=2, name=f"cout0_{it}")
                        nc.sync.dma_start(cin0[:], mloc[:, 0:8, :])
                        nc.gpsimd.collective_compute(
                            "AllGather", ALU.bypass, replica_groups=GROUPS,
                            ins=[cin0[:].opt()], outs=[cout0[:].opt()])
                        nc.sync.dma_start(mfA_n[:, 0:8, :], cout0[0:128, :, :])
                        nc.sync.dma_start(mfA_n[:, 8:16, :],
                                          cout0[128:256, :, :])
                    if rb == 3:
                        cin1 = dr.tile([128, 8, H], fp8, tag="cin1",
                                       bufs=2, name=f"cin1_{it}")
                        cout1 = dr.tile([256, 8, H], fp8, tag="cout1",
                                        bufs=2, name=f"cout1_{it}")
                        nc.sync.dma_start(cin1[:], mloc[:, 8:16, :])
                        nc.gpsimd.collective_compute(
                            "AllGather", ALU.bypass, replica_groups=GROUPS,
                            ins=[cin1[:].opt()], outs=[cout1[:].opt()])
                        nc.sync.dma_start(mfB_n[:, 0:8, :], cout1[0:128, :, :])
                        nc.sync.dma_start(mfB_n[:, 8:16, :],
                                          cout1[128:256, :, :])
                else:
                    v1p = ps.tile([H, RB], f32, tag="pmlp", bufs=1,
                                  name=f"v1p_{rb}")
                    nc.tensor.matmul(v1p[:], vw1gt[:], hnLoc[:, rbsl(rb)],
                                     start=True, stop=True)
                    v1s = wk.tile([H, RB], bf16, tag="v1s", bufs=2,
                                  name=f"v1s_{rb}")
                    nc.vector.tensor_scalar(v1s[:], v1p[:], vb1c[:], 0.0,
                                            ALU.add, ALU.max)
                    v2p = ps.tile([H, RB], f32, tag="pmlp", bufs=1,
                                  name=f"v2p_{rb}")
                    nc.tensor.matmul(v2p[:], vw2t[:], v1s[:],
                                     start=True, stop=True)
                    v2s = wk.tile([H, RB], bf16, tag="v2s", bufs=2,
                                  name=f"v2s_{rb}")
                    nc.vector.tensor_scalar(v2s[:], v2p[:], vb2c[:], 0.0,
                                            ALU.add, ALU.max)
                    vop = ps.tile([1, RB], f32, tag="pmlp", bufs=1,
                                  name=f"vop_{rb}")
                    nc.tensor.matmul(vop[:], vw3t[:], v2s[:],
                                     start=True, stop=True)
                    vos = wk.tile([1, RB], f32, tag="vos", bufs=2,
                                  name=f"vos_{rb}")
                    nc.vector.tensor_copy(vos[:], vop[:])
                    nc.sync.dma_start(votes_out[:, rbsl(rb)], vos[:])

            for it in range(1, iters + 1):
                mfA, mfB = mf_cur
                if it < iters:
                    mloc = st.tile([128, NLCH, H], fp8, tag="mloc", bufs=2,
                                   name=f"mloc_{it}")
                    mfA_n = st.tile([128, NLCH, H], fp8, tag="mfA", bufs=2,
                                    name=f"mfA_{it}")
                    mfB_n = st.tile([128, NLCH, H], fp8, tag="mfB", bufs=2,
                                    name=f"mfB_{it}")
                else:
                    mloc = mfA_n = mfB_n = None
                gps_q = []
                for rb in range(NRB):
                    # ---- A_rb: bmm for this row block ----
                    mp = ps.tile([H, RB], f32, tag="pmsg", bufs=2,
                                 name=f"mp_{it}_{rb}")
                    if it == 1:
                        nc.tensor.matmul(mp[:], m0_sb[:], rs_sb[:, rbsl(rb)],
                                         start=True, stop=True)
                    else:
                        nc.tensor.matmul(mp[:], b3r[:], rs_sb[:, rbsl(rb)],
                                         start=True, stop=False)
                        for c in ORDER_A + ORDER_B:
                            nc.tensor.matmul(mp[:], pair_lhsT(c, mfA, mfB),
                                             xt[c][:, :, rbsl(rb)],
                                             start=False,
                                             stop=(c == ORDER_B[-1]),
                                             perf_mode=PM.DoubleRow)
                    # msgb on the scalar engine (frees the pmsg bank fast)
                    nc.scalar.copy(grhs[rb][:, 0, :], mp[:])
                    g4 = []
                    for g in range(4):
                        gp = ps.tile([H, RB], f32, tag="pgat", bufs=3,
                                     name=f"gp_{it}_{rb}_{g}")
                        nc.tensor.matmul(gp[:], wpair[:, :, g * H:(g + 1) * H],
                                         grhs[rb][:], start=True, stop=True,
                                         perf_mode=PM.DoubleRow)
                        g4.append(gp)
                    gps_q.append(g4)
                    if rb >= 1:
                        tail_pe(it, rb - 1, mloc, mfA_n, mfB_n)
                    tail_chain(it, rb, gps_q[rb])
                tail_pe(it, 3, mloc, mfA_n, mfB_n)
                if it < iters:
                    mf_cur = (mfA_n, mfB_n)

    nc.compile()
    return nc


_NC_CACHE = {}


def _get_module():
    key = (N, ITERS)
    if key not in _NC_CACHE:
        _NC_CACHE[key] = build_module(ITERS)
    return _NC_CACHE[key]


def _host_prep(inputs):
    """Fold weights, run init MLP, build per-core in_maps."""
    g = lambda s: np.asarray(inputs[s], np.float32)
    x = g("x")
    k, n = g("k"), g("n")

    nk = np.stack([k, n], 1)
    a = np.maximum(nk @ g("init_w1").T + g("init_b1"), 0)
    a = np.maximum(a @ g("init_w2").T + g("init_b2"), 0)
    init0 = a @ g("init_w3").T + g("init_b3")          # [B, H]

    ln_g, ln_b = g("ln_g"), g("ln_b")
    mu0 = init0.mean(1, keepdims=True)
    var0 = init0.var(1, keepdims=True)
    embed0 = (init0 - mu0) / np.sqrt(var0 + EPS) * ln_g + ln_b
    t = np.maximum(embed0 @ g("msg_w1").T + g("msg_b1"), 0)
    t = np.maximum(t @ g("msg_w2").T + g("msg_b2"), 0)
    m0eff = t @ g("msg_w3").T + g("msg_b3")            # [B, H]

    bsum = (g("lstm_bih") + g("lstm_bhh")).reshape(4, H).T  # [H, 4] i,f,g,o
    bg2 = bsum.copy()
    bg2[:, 0] *= 0.5
    bg2[:, 1] *= 0.5
    bg2[:, 3] *= 0.5

    wpair = np.empty((H, 2, 4 * H), np.float32)
    wpair[:, 0, :] = WIH_S * g("lstm_wih").T
    wpair[:, 1, :] = WHH_S * g("lstm_whh").T

    com = {
        "w1gt": (g("msg_w1") * ln_g[None, :]).T.astype(BF),
        "w2t": g("msg_w2").T.astype(BF),
        "w3t4": (M_S * g("msg_w3")).T.astype(BF),
        "vw1gt": (g("vote_w1") * ln_g[None, :]).T.astype(BF),
        "vw2t": g("vote_w2").T.astype(BF),
        "vw3t": g("vote_w3").T.astype(BF),              # [H, 1]
        "wpair": wpair.astype(F8),
        "bg2": np.ascontiguousarray(bg2, np.float32),
        "b1c": (g("msg_w1") @ ln_b + g("msg_b1")).reshape(H, 1).astype(np.float32),
        "b2c": g("msg_b2").reshape(H, 1).astype(np.float32),
        "vb1c": (g("vote_w1") @ ln_b + g("vote_b1")).reshape(H, 1).astype(np.float32),
        "vb2c": g("vote_b2").reshape(H, 1).astype(np.float32),
        "b3r": (M_S * g("msg_b3")).reshape(1, H).astype(BF),
        "id8": np.eye(H, dtype=F8),
        "id16": np.eye(H, dtype=BF),
    }

    in_maps = []
    for core in range(NCORES):
        b = core // 2
        r0 = (core % 2) * R
        xs = x[b][r0:r0 + R, :]                        # [R, N] local rows
        x8 = xs.astype(F8)
        m = dict(com)
        # xtdr[c*128+p, kp, row] = x8[row, 256c + 128kp + p]
        xt8 = np.ascontiguousarray(x8.T)               # [N, R]
        m["xtdr"] = np.ascontiguousarray(
            xt8.reshape(NPAIR, 2, 128, R).transpose(0, 2, 1, 3)
        ).reshape(NPAIR * 128, 2, R)
        m["rs"] = x8.astype(np.float32).sum(1).reshape(1, R).astype(BF)
        m["h0"] = np.ascontiguousarray(
            np.broadcast_to(2.0 * init0[b][:, None], (H, R))).astype(F8)
        m["m0"] = (M_S * m0eff[b]).reshape(1, H).astype(BF)
        in_maps.append(m)
    return in_maps


def kernel(**inputs):
    nc = _get_module()
    in_maps = _host_prep(inputs)
    res = run_bass_kernel_spmd(nc, in_maps, core_ids=list(range(NCORES)))
    mask = np.asarray(inputs["mask"], np.float64)
    vb3 = float(np.asarray(inputs["vote_b3"], np.float64).reshape(-1)[0])
    out = np.zeros(B, np.float32)
    for b in range(B):
        votes = np.concatenate([
            res.results[2 * b]["votes"].reshape(-1),
            res.results[2 * b + 1]["votes"].reshape(-1),
        ]).astype(np.float64) + vb3
        s = float((votes * mask[b]).sum())
        out[b] = 1.0 / (1.0 + np.exp(-s))
    return out


# revision 20
# speedup vs baseline: 1.0712x; 1.0712x over previous
"""Trainium2 Bass kernel for NeuroISNet GNN message passing (v2).

Strategy (8 NeuronCores, one trn2 chip):
  - Batch b -> core pair (2b, 2b+1); each core owns 2048 of 4096 node rows.
  - Dominant bmm msg = x @ m runs in fp8e4 with perf_mode=DoubleRow:
    x^T resident in SBUF as 16 pair-chunks [128, 2, 2048] (8 MB), m in fp8
    pair tiles; 256-deep contraction per matmul, f32 PSUM accumulation.
  - LSTM gates also fp8 DoubleRow: one matmul per gate computes
    Wih@msg + Whh@h via the two planes of a shared [128, 2, 512] rhs tile
    (plane0 = 4*msg, plane1 = 2*h). Weights stored x16/x32 so fp8 keeps
    mantissa; activation `scale` undoes the folding.
  - All-tanh LSTM: sigmoid(x) = (tanh(x/2)+1)/2, cell state stored as
    C = 2c and hidden as Hs = 2h (LN is scale-invariant; Whh absorbs the
    0.5) -> the scalar engine runs ONLY Tanh, zero act-table reloads.
  - LayerNorm per 128-row tile via PE transpose + bn_stats; 1/sigma via
    quake-rsqrt (bitcast + one Newton step) on the vector engine.
  - msg-MLP runs on LOCAL 2048 nodes only; the m output (fp8) is
    AllGathered between the core pair as two chunked collectives per
    iteration, issued mid-iteration so the bmm of the next iteration
    never waits. m-chunk availability is rank-symmetric (both halves of
    each gather land together), so the SPMD program has no rank branches.
  - m3's bias enters the bmm as a rank-1 matmul with x row-sums.
  - Iteration 1 exploits identical initial rows: msg1 = m0 (x) rowsums.
  - Host does only O(B*H^2) prep: folding weights, init MLP, transposes.
"""

import numpy as np
import ml_dtypes

import concourse.bass as bass
import concourse.mybir as mybir
import concourse.tile as tile
from concourse import bacc
from concourse.bass_utils import run_bass_kernel_spmd

BF = ml_dtypes.bfloat16
F8 = ml_dtypes.float8_e4m3
bf16 = mybir.dt.bfloat16
f32 = mybir.dt.float32
fp8 = mybir.dt.float8e4
u32 = mybir.dt.uint32

B, N, H, ITERS = 4, 4096, 128, 8
EPS = 1e-5
NCORES = 8
R = N // 2               # local rows per core
NRB = 4                  # row blocks
RB = R // NRB            # 512
NPAIR = 16               # global DoubleRow pairs (4096 / 256)
NLCH = 16                # local 128-chunks (2048 / 128)
GROUPS = [[0, 1], [2, 3], [4, 5], [6, 7]]

AF = mybir.ActivationFunctionType
ALU = mybir.AluOpType
PM = mybir.MatmulPerfMode

WIH_S = 16.0             # lstm_wih stored x16 (fp8 range)
WHH_S = 32.0             # lstm_whh stored x32 (fp8 range + absorbs h = Hs/2)
M_S = 4.0                # m stored x4 (fp8 subnormal floor)
GS = WIH_S * M_S         # psum gate scale (= WHH_S * 2)
QUAKE_K = 0x5F3759DF


def build_module(iters=ITERS):
    nc = bacc.Bacc("TRN2", target_bir_lowering=False, debug=False,
                   num_devices=NCORES)

    din = lambda name, shape, dt: nc.dram_tensor(name, shape, dt,
                                                 kind="ExternalInput")
    xtdr_in = din("xtdr", [NPAIR * 128, 2, R], fp8)
    h0_in = din("h0", [H, R], fp8)              # Hs = 2*h0
    rs_in = din("rs", [1, R], bf16)
    m0_in = din("m0", [1, H], bf16)             # 4*m0eff
    b3r_in = din("b3r", [1, H], bf16)           # 4*msg_b3
    w1gt_in = din("w1gt", [H, H], bf16)
    w2t_in = din("w2t", [H, H], bf16)
    w3t4_in = din("w3t4", [H, H], bf16)         # 4*msg_w3.T
    vw1gt_in = din("vw1gt", [H, H], bf16)
    vw2t_in = din("vw2t", [H, H], bf16)
    vw3t_in = din("vw3t", [H, 1], bf16)
    wpair_in = din("wpair", [H, 2, 4 * H], fp8)  # [16*wih.T | 32*whh.T]
    bg2_in = din("bg2", [H, 4], f32)            # pre-scaled gate biases
    b1c_in = din("b1c", [H, 1], f32)
    b2c_in = din("b2c", [H, 1], f32)
    vb1c_in = din("vb1c", [H, 1], f32)
    vb2c_in = din("vb2c", [H, 1], f32)
    id8_in = din("id8", [H, H], fp8)
    id16_in = din("id16", [H, H], bf16)

    votes_out = nc.dram_tensor("votes", [1, R], f32, kind="ExternalOutput")

    rbsl = lambda rb: slice(rb * RB, (rb + 1) * RB)

    with tile.TileContext(nc) as tc:
        with tc.tile_pool(name="const", bufs=1) as cp, \
             tc.tile_pool(name="state", bufs=1) as st, \
             tc.tile_pool(name="work", bufs=1) as wk, \
             tc.tile_pool(name="ps", bufs=1, space="PSUM") as ps, \
             tc.tile_pool(name="dram", bufs=1, space="DRAM") as dr:

            # ---- constants ----
            def cload(inp, shape, dt, tag):
                t = cp.tile(shape, dt, tag=tag, name=tag)
                nc.sync.dma_start(t[:], inp[:])
                return t

            w1gt = cload(w1gt_in, [H, H], bf16, "w1gt")
            w2t = cload(w2t_in, [H, H], bf16, "w2t")
            w3t4 = cload(w3t4_in, [H, H], bf16, "w3t4")
            vw1gt = cload(vw1gt_in, [H, H], bf16, "vw1gt")
            vw2t = cload(vw2t_in, [H, H], bf16, "vw2t")
            vw3t = cload(vw3t_in, [H, 1], bf16, "vw3t")
            wpair = cload(wpair_in, [H, 2, 4 * H], fp8, "wpair")
            bg2 = cload(bg2_in, [H, 4], f32, "bg2")
            b1c = cload(b1c_in, [H, 1], f32, "b1c")
            b2c = cload(b2c_in, [H, 1], f32, "b2c")
            vb1c = cload(vb1c_in, [H, 1], f32, "vb1c")
            vb2c = cload(vb2c_in, [H, 1], f32, "vb2c")
            id8 = cload(id8_in, [H, H], fp8, "id8")
            id16 = cload(id16_in, [H, H], bf16, "id16")
            rs_sb = cload(rs_in, [1, R], bf16, "rs")
            m0_sb = cload(m0_in, [1, H], bf16, "m0")
            b3r = cload(b3r_in, [1, H], bf16, "b3r")
            qk = cp.tile([H, 4], u32, tag="qk", name="qk")
            nc.vector.memset(qk[:], QUAKE_K)

            # ---- state ----
            grhs = []                 # gates rhs: plane0 = 4*msg, plane1 = Hs
            for rb in range(NRB):
                t = st.tile([H, 2, RB], fp8, tag=f"grhs{rb}", name=f"grhs{rb}")
                nc.sync.dma_start(t[:, 1, :], h0_in[:, rbsl(rb)])
                grhs.append(t)
            cst = []                  # C = 2c, bf16
            for rb in range(NRB):
                t = st.tile([H, RB], bf16, tag=f"c{rb}", name=f"c{rb}")
                nc.vector.memset(t[:], 0.0)
                cst.append(t)
            hnLoc = st.tile([H, R], bf16, tag="hnLoc", name="hnLoc")

            # ---- resident x^T pair-chunks, in bmm consumption order ----
            ORDER_A = [0, 1, 2, 3, 8, 9, 10, 11]
            ORDER_B = [4, 5, 6, 7, 12, 13, 14, 15]
            xt = [None] * NPAIR
            for c in ORDER_A + ORDER_B:
                t = st.tile([128, 2, R], fp8, tag=f"xt{c}", name=f"xt{c}")
                nc.sync.dma_start(t[:], xtdr_in[c * 128:(c + 1) * 128, :, :])
                xt[c] = t

            def pair_lhsT(c, mfA, mfB):
                """lhsT [128, 2, H] for global pair c from gathered m tiles."""
                if c < 8:
                    t, s = (mfA, 2 * c) if c < 4 else (mfB, 2 * (c - 4))
                else:
                    t, s = (mfA, 8 + 2 * (c - 8)) if c < 12 else \
                           (mfB, 8 + 2 * (c - 12))
                return t[:, s:s + 2, :]

            mf_cur = (None, None)

            # warm up the collective path while input DMAs stream
            cinw = dr.tile([1, 16], fp8, tag="cinw", name="cinw")
            coutw = dr.tile([2, 16], fp8, tag="coutw", name="coutw")
            nc.sync.dma_start(cinw[:], id8[0:1, 0:16])
            nc.gpsimd.collective_compute(
                "AllGather", ALU.bypass, replica_groups=GROUPS,
                ins=[cinw[:].opt()], outs=[coutw[:].opt()])

            # ================= main loop =================
            # rb-major software pipeline: each rb's bmm block (A_rb) is
            # emitted between the previous rb's dependent tail work, so the
            # PE always has ready matmuls while scalar/vector chains run.
            mf_cur = (None, None)

            def tail_chain(it, rb, gps):
                """LSTM tail: scalar tanhs + gpsimd/vector state update."""
                gi, gf, gg, go = gps
                ti = wk.tile([H, RB], bf16, tag="ti", bufs=2,
                             name=f"ti_{it}_{rb}")
                nc.scalar.activation(ti[:], gi[:], AF.Tanh,
                                     bias=bg2[:, 0:1], scale=0.5 / GS)
                tf = wk.tile([H, RB], bf16, tag="tf", bufs=2,
                             name=f"tf_{it}_{rb}")
                nc.scalar.activation(tf[:], gf[:], AF.Tanh,
                                     bias=bg2[:, 1:2], scale=0.5 / GS)
                tg = wk.tile([H, RB], bf16, tag="tg", bufs=2,
                             name=f"tg_{it}_{rb}")
                nc.scalar.activation(tg[:], gg[:], AF.Tanh,
                                     bias=bg2[:, 2:3], scale=1.0 / GS)
                to = wk.tile([H, RB], bf16, tag="to", bufs=2,
                             name=f"to_{it}_{rb}")
                nc.scalar.activation(to[:], go[:], AF.Tanh,
                                     bias=bg2[:, 3:4], scale=0.5 / GS)
                # C' = 0.5*(tf+1)*C + (ti+1)*tg   (C = 2c)
                uu = wk.tile([H, RB], bf16, tag="uu", bufs=2,
                             name=f"uu_{it}_{rb}")
                nc.vector.scalar_tensor_tensor(
                    uu[:], tf[:], 1.0, cst[rb][:], ALU.add, ALU.mult)
                vv = wk.tile([H, RB], bf16, tag="vv", bufs=2,
                             name=f"vv_{it}_{rb}")
                nc.vector.scalar_tensor_tensor(
                    vv[:], ti[:], 1.0, tg[:], ALU.add, ALU.mult)
                nc.vector.scalar_tensor_tensor(
                    cst[rb][:], uu[:], 0.5, vv[:], ALU.mult, ALU.add)
                tnc = wk.tile([H, RB], bf16, tag="tnc", bufs=2,
                              name=f"tnc_{it}_{rb}")
                nc.scalar.activation(tnc[:], cst[rb][:], AF.Tanh, scale=0.5)
                # Hs = (to+1)*tanh(c)   (= 2h)
                nc.vector.scalar_tensor_tensor(
                    grhs[rb][:, 1, :], to[:], 1.0, tnc[:], ALU.add, ALU.mult)

            def tail_pe(it, rb, mloc, mfA_n, mfB_n):
                """LN + local msg-MLP (or vote) for one row block."""
                # fp8 transpose must write with element step 2
                trp = ps.tile([128, 2 * RB], fp8, tag="ptrp", bufs=1,
                              name=f"trp_{it}_{rb}")
                tsl8 = lambda t: slice(2 * t * 128, 2 * (t + 1) * 128, 2)
                mv = wk.tile([128, 8], f32, tag="mv", bufs=2,
                             name=f"mv_{it}_{rb}")
                for t in range(4):
                    nc.tensor.transpose(trp[:, tsl8(t)],
                                        grhs[rb][:, 1, t * 128:(t + 1) * 128],
                                        id8[:])
                    stt = wk.tile([128, 6], f32, tag="st6", bufs=4,
                                  name=f"st_{it}_{rb}_{t}")
                    nc.vector.bn_stats(stt[:], trp[:, tsl8(t)])
                    nc.vector.bn_aggr(mv[:, 2 * t:2 * t + 2], stt[:])
                # quake rsqrt(var + eps), one Newton step
                ve = wk.tile([128, 4], f32, tag="ve", bufs=2,
                             name=f"ve_{it}_{rb}")
                nc.vector.tensor_scalar(ve[:], mv[:, 1:8:2], EPS, None, ALU.add)
                sh = wk.tile([128, 4], u32, tag="sh", bufs=2,
                             name=f"sh_{it}_{rb}")
                nc.vector.tensor_scalar(sh[:], ve[:].bitcast(u32), 1, None,
                                        ALU.logical_shift_right)
                y0 = wk.tile([128, 4], u32, tag="y0", bufs=2,
                             name=f"y0_{it}_{rb}")
                nc.vector.tensor_tensor(y0[:], qk[:], sh[:], ALU.subtract)
                # apply + transpose back
                hnr = wk.tile([128, RB], bf16, tag="hnr", bufs=2,
                              name=f"hnr_{it}_{rb}")
                for t in range(4):
                    nc.vector.tensor_scalar(
                        hnr[:, t * 128:(t + 1) * 128], trp[:, tsl8(t)],
                        mv[:, 2 * t:2 * t + 1], y0[:, t:t + 1].bitcast(f32),
                        ALU.subtract, ALU.mult)
                cbk = ps.tile([128, RB], bf16, tag="pbk", bufs=1,
                              name=f"cbk_{it}_{rb}")
                for t in range(4):
                    nc.tensor.transpose(cbk[:, t * 128:(t + 1) * 128],
                                        hnr[:, t * 128:(t + 1) * 128],
                                        id16[:])
                nc.scalar.copy(hnLoc[:, rbsl(rb)], cbk[:])

                if it < iters:
                    m1p = ps.tile([H, RB], f32, tag="pmlp", bufs=1,
                                  name=f"m1p_{it}_{rb}")
                    nc.tensor.matmul(m1p[:], w1gt[:], hnLoc[:, rbsl(rb)],
                                     start=True, stop=True)
                    m1s = wk.tile([H, RB], bf16, tag="m1s", bufs=2,
                                  name=f"m1s_{it}_{rb}")
                    nc.vector.tensor_scalar(m1s[:], m1p[:], b1c[:], 0.0,
                                            ALU.add, ALU.max)
                    m2p = ps.tile([H, RB], f32, tag="pmlp", bufs=1,
                                  name=f"m2p_{it}_{rb}")
                    nc.tensor.matmul(m2p[:], w2t[:], m1s[:],
                                     start=True, stop=True)
                    m2s = wk.tile([H, RB], bf16, tag="m2s", bufs=2,
                                  name=f"m2s_{it}_{rb}")
                    nc.vector.tensor_scalar(m2s[:], m2p[:], b2c[:], 0.0,
                                            ALU.add, ALU.max)
                    m3p = ps.tile([H, RB], f32, tag="pmlp", bufs=1,
                                  name=f"m3p_{it}_{rb}")
                    for t in range(4):
                        nc.tensor.matmul(m3p[:, t * 128:(t + 1) * 128],
                                         m2s[:, t * 128:(t + 1) * 128],
                                         w3t4[:], start=True, stop=True)
                    nc.scalar.copy(mloc[:, rb * 4:(rb + 1) * 4, :], m3p[:])
                    if rb == 1:
                        cin0 = dr.tile([128, 8, H], fp8, tag="cin0",
                                       bufs=2, name=f"cin0_{it}")
                        cout0 = dr.tile([256, 8, H], fp8, tag="cout0",
                                        bufs=2, name=f"cout0_{it}")
                        nc.sync.dma_start(cin0[:], mloc[:, 0:8, :])
                        nc.gpsimd.collective_compute(
                            "AllGather", ALU.bypass, replica_groups=GROUPS,
                            ins=[cin0[:].opt()], outs=[cout0[:].opt()])
                        nc.sync.dma_start(mfA_n[:, 0:8, :], cout0[0:128, :, :])
                        nc.sync.dma_start(mfA_n[:, 8:16, :],
                                          cout0[128:256, :, :])
                    if rb == 3:
                        cin1 = dr.tile([128, 8, H], fp8, tag="cin1",
                                       bufs=2, name=f"cin1_{it}")
                        cout1 = dr.tile([256, 8, H], fp8, tag="cout1",
                                        bufs=2, name=f"cout1_{it}")
                        nc.sync.dma_start(cin1[:], mloc[:, 8:16, :])
                        nc.gpsimd.collective_compute(
                            "AllGather", ALU.bypass, replica_groups=GROUPS,
                            ins=[cin1[:].opt()], outs=[cout1[:].opt()])
                        nc.sync.dma_start(mfB_n[:, 0:8, :], cout1[0:128, :, :])
                        nc.sync.dma_start(mfB_n[:, 8:16, :],
                                          cout1[128:256, :, :])
                else:
                    v1p = ps.tile([H, RB], f32, tag="pmlp", bufs=1,
                                  name=f"v1p_{rb}")
                    nc.tensor.matmul(v1p[:], vw1gt[:], hnLoc[:, rbsl(rb)],
                                     start=True, stop=True)
                    v1s = wk.tile([H, RB], bf16, tag="v1s", bufs=2,
                                  name=f"v1s_{rb}")
                    nc.vector.tensor_scalar(v1s[:], v1p[:], vb1c[:], 0.0,
                                            ALU.add, ALU.max)
                    v2p = ps.tile([H, RB], f32, tag="pmlp", bufs=1,
                                  name=f"v2p_{rb}")
                    nc.tensor.matmul(v2p[:], vw2t[:], v1s[:],
                                     start=True, stop=True)
                    v2s = wk.tile([H, RB], bf16, tag="v2s", bufs=2,
                                  name=f"v2s_{rb}")
                    nc.vector.tensor_scalar(v2s[:], v2p[:], vb2c[:], 0.0,
                                            ALU.add, ALU.max)
                    vop = ps.tile([1, RB], f32, tag="pmlp", bufs=1,
                                  name=f"vop_{rb}")
                    nc.tensor.matmul(vop[:], vw3t[:], v2s[:],
                                     start=True, stop=True)
                    vos = wk.tile([1, RB], f32, tag="vos", bufs=2,
                                  name=f"vos_{rb}")
                    nc.vector.tensor_copy(vos[:], vop[:])
                    nc.sync.dma_start(votes_out[:, rbsl(rb)], vos[:])

            for it in range(1, iters + 1):
                mfA, mfB = mf_cur
                if it < iters:
                    mloc = st.tile([128, NLCH, H], fp8, tag="mloc", bufs=2,
                                   name=f"mloc_{it}")
                    mfA_n = st.tile([128, NLCH, H], fp8, tag="mfA", bufs=2,
                                    name=f"mfA_{it}")
                    mfB_n = st.tile([128, NLCH, H], fp8, tag="mfB", bufs=2,
                                    name=f"mfB_{it}")
                else:
                    mloc = mfA_n = mfB_n = None
                mps = {}

                def bmm_head(rbs, it=it, mfA=mfA, mfB=mfB, mps=mps):
                    """b3r rank-1 + ORDER_A pairs (gather-a fed) for 2 rbs."""
                    for rb in rbs:
                        mps[rb] = ps.tile([H, RB], f32, tag="pmsg", bufs=2,
                                          name=f"mp_{it}_{rb}")
                        if it == 1:
                            nc.tensor.matmul(mps[rb][:], m0_sb[:],
                                             rs_sb[:, rbsl(rb)],
                                             start=True, stop=True)
                        else:
                            nc.tensor.matmul(mps[rb][:], b3r[:],
                                             rs_sb[:, rbsl(rb)],
                                             start=True, stop=False)
                    if it > 1:
                        for c in ORDER_A:
                            lt = pair_lhsT(c, mfA, mfB)
                            for rb in rbs:
                                nc.tensor.matmul(mps[rb][:], lt,
                                                 xt[c][:, :, rbsl(rb)],
                                                 start=False, stop=False,
                                                 perf_mode=PM.DoubleRow)

                def bmm_tail_gates(rb, it=it, mfA=mfA, mfB=mfB, mps=mps):
                    """ORDER_B pairs (gather-b fed) + msgb + gate matmuls."""
                    if it > 1:
                        for c in ORDER_B:
                            nc.tensor.matmul(mps[rb][:],
                                             pair_lhsT(c, mfA, mfB),
                                             xt[c][:, :, rbsl(rb)],
                                             start=False,
                                             stop=(c == ORDER_B[-1]),
                                             perf_mode=PM.DoubleRow)
                    nc.scalar.copy(grhs[rb][:, 0, :], mps[rb][:])
                    g4 = []
                    for g in range(4):
                        gp = ps.tile([H, RB], f32, tag="pgat", bufs=3,
                                     name=f"gp_{it}_{rb}_{g}")
                        nc.tensor.matmul(gp[:], wpair[:, :, g * H:(g + 1) * H],
                                         grhs[rb][:], start=True, stop=True,
                                         perf_mode=PM.DoubleRow)
                        g4.append(gp)
                    return g4

                bmm_head([0, 1])
                g4 = bmm_tail_gates(0)
                tail_chain(it, 0, g4)
                g4 = bmm_tail_gates(1)
                tail_chain(it, 1, g4)
                bmm_head([2, 3])
                tail_pe(it, 0, mloc, mfA_n, mfB_n)
                g4 = bmm_tail_gates(2)
                tail_chain(it, 2, g4)
                tail_pe(it, 1, mloc, mfA_n, mfB_n)
                g4 = bmm_tail_gates(3)
                tail_chain(it, 3, g4)
                tail_pe(it, 2, mloc, mfA_n, mfB_n)
                tail_pe(it, 3, mloc, mfA_n, mfB_n)
                if it < iters:
                    mf_cur = (mfA_n, mfB_n)

    nc.compile()
    return nc


_NC_CACHE = {}


def _get_module():
    key = (N, ITERS)
    if key not in _NC_CACHE:
        _NC_CACHE[key] = build_module(ITERS)
    return _NC_CACHE[key]


def _host_prep(inputs):
    """Fold weights, run init MLP, build per-core in_maps."""
    g = lambda s: np.asarray(inputs[s], np.float32)
    x = g("x")
    k, n = g("k"), g("n")

    nk = np.stack([k, n], 1)
    a = np.maximum(nk @ g("init_w1").T + g("init_b1"), 0)
    a = np.maximum(a @ g("init_w2").T + g("init_b2"), 0)
    init0 = a @ g("init_w3").T + g("init_b3")          # [B, H]

    ln_g, ln_b = g("ln_g"), g("ln_b")
    mu0 = init0.mean(1, keepdims=True)
    var0 = init0.var(1, keepdims=True)
    embed0 = (init0 - mu0) / np.sqrt(var0 + EPS) * ln_g + ln_b
    t = np.maximum(embed0 @ g("msg_w1").T + g("msg_b1"), 0)
    t = np.maximum(t @ g("msg_w2").T + g("msg_b2"), 0)
    m0eff = t @ g("msg_w3").T + g("msg_b3")            # [B, H]

    bsum = (g("lstm_bih") + g("lstm_bhh")).reshape(4, H).T  # [H, 4] i,f,g,o
    bg2 = bsum.copy()
    bg2[:, 0] *= 0.5
    bg2[:, 1] *= 0.5
    bg2[:, 3] *= 0.5

    wpair = np.empty((H, 2, 4 * H), np.float32)
    wpair[:, 0, :] = WIH_S * g("lstm_wih").T
    wpair[:, 1, :] = WHH_S * g("lstm_whh").T

    com = {
        "w1gt": (g("msg_w1") * ln_g[None, :]).T.astype(BF),
        "w2t": g("msg_w2").T.astype(BF),
        "w3t4": (M_S * g("msg_w3")).T.astype(BF),
        "vw1gt": (g("vote_w1") * ln_g[None, :]).T.astype(BF),
        "vw2t": g("vote_w2").T.astype(BF),
        "vw3t": g("vote_w3").T.astype(BF),              # [H, 1]
        "wpair": wpair.astype(F8),
        "bg2": np.ascontiguousarray(bg2, np.float32),
        "b1c": (g("msg_w1") @ ln_b + g("msg_b1")).reshape(H, 1).astype(np.float32),
        "b2c": g("msg_b2").reshape(H, 1).astype(np.float32),
        "vb1c": (g("vote_w1") @ ln_b + g("vote_b1")).reshape(H, 1).astype(np.float32),
        "vb2c": g("vote_b2").reshape(H, 1).astype(np.float32),
        "b3r": (M_S * g("msg_b3")).reshape(1, H).astype(BF),
        "id8": np.eye(H, dtype=F8),
        "id16": np.eye(H, dtype=BF),
    }

    in_maps = []
    for core in range(NCORES):
        b = core // 2
        r0 = (core % 2) * R
        xs = x[b][r0:r0 + R, :]                        # [R, N] local rows
        x8 = xs.astype(F8)
        m = dict(com)
        # xtdr[c*128+p, kp, row] = x8[row, 256c + 128kp + p]
        xt8 = np.ascontiguousarray(x8.T)               # [N, R]
        m["xtdr"] = np.ascontiguousarray(
            xt8.reshape(NPAIR, 2, 128, R).transpose(0, 2, 1, 3)
        ).reshape(NPAIR * 128, 2, R)
        m["rs"] = x8.astype(np.float32).sum(1).reshape(1, R).astype(BF)
        m["h0"] = np.ascontiguousarray(
            np.broadcast_to(2.0 * init0[b][:, None], (H, R))).astype(F8)
        m["m0"] = (M_S * m0eff[b]).reshape(1, H).astype(BF)
        in_maps.append(m)
    return in_maps


def kernel(**inputs):
    nc = _get_module()
    in_maps = _host_prep(inputs)
    res = run_bass_kernel_spmd(nc, in_maps, core_ids=list(range(NCORES)))
    mask = np.asarray(inputs["mask"], np.float64)
    vb3 = float(np.asarray(inputs["vote_b3"], np.float64).reshape(-1)[0])
    out = np.zeros(B, np.float32)
    for b in range(B):
        votes = np.concatenate([
            res.results[2 * b]["votes"].reshape(-1),
            res.results[2 * b + 1]["votes"].reshape(-1),
        ]).astype(np.float64) + vb3
        s = float((votes * mask[b]).sum())
        out[b] = 1.0 / (1.0 + np.exp(-s))
    return out
